# revision 1
# baseline (speedup 1.0000x reference)
"""Trainium2 Bass kernel for nn_ConsistencyLoss.

Strategy: the reference's depthwise complex conv (per-frequency kernel width
1023 along the 1025 frames) is replaced by shared-matrix DFTs:

  loss*B*T = sum |C[b,n,t]|^2,  C[b,n,:] = central-1025 samples of K[n] (*) H_full[b,n,:]

  1. STFT as matmul: H[t, n] = frames(t,:) @ (W * DFT_512)      (two-sided bins)
  2. ghat[f, n] = Khat[n, f] * sum_t H[t, n] e^{-2pi i f t / L}  (L = 1536)
  3. C[t', n] = sum_f ghat[f, n] e^{+2pi i f (t'+511) / L}
  4. accumulate |C|^2

L = 1536 >= 2047 - 511 makes the circular conv exact on the central samples.
All heavy stages are PE matmuls with shared (host-precomputed) DFT matrices.

Sharding: 8 cores = 4 batch rows x 2 halves of the 512 frequency bins.
Per-core output is a [128, 18] partial-sum tile; host sums and normalizes.
"""
import numpy as np
import ml_dtypes

N = 512
R = 128
Q = 4
T = 1025
TP = 1152            # frames padded to 9*128
LDFT = 1536          # 12*128
NB = 256             # bins per core
FCH = 12             # f chunks of 128
TCH = 9              # t' (and t) chunks of 128
B = 4


# ---------------------------------------------------------------- host prep
def _build_host_constants(window, alpha_real, alpha_imag):
    alpha = alpha_real.astype(np.complex128) + 1j * alpha_imag.astype(np.complex128)
    n_idx = np.arange(N)
    q_idx = np.arange(-(Q - 1), Q)
    phase = np.exp(1j * (2 * np.pi / N) * np.outer(n_idx, q_idx))
    K = phase @ alpha                                 # (512, 1023)
    Khat = np.fft.fft(K, LDFT, axis=1) / LDFT         # (512, 1536)

    W = window.astype(np.float64)
    j = np.arange(N)
    wdfts, khats = [], []
    for half in range(2):
        ns = np.arange(half * NB, half * NB + NB)
        ang = 2 * np.pi * np.outer(j, ns) / N
        wd = np.concatenate([
            W[:, None] * np.cos(ang),
            -W[:, None] * np.sin(ang),
            -W[:, None] * np.cos(ang),
        ], axis=1)
        wdfts.append(wd.astype(np.float32))           # (512, 768)
        ks = Khat[half * NB: half * NB + NB]          # (256, 1536)
        kh = np.concatenate([ks.real.T, ks.imag.T, -ks.imag.T], axis=1)
        khats.append(kh.astype(np.float32))           # (1536, 768)

    t_idx = np.arange(TP)
    f_idx = np.arange(LDFT)
    angl = 2 * np.pi * np.outer(t_idx, f_idx) / LDFT
    C1, S1 = np.cos(angl), np.sin(angl)
    e1 = np.empty((FCH, TP, 256), dtype=ml_dtypes.bfloat16)
    for fi in range(FCH):
        e1[fi, :, 0:128] = C1[:, fi * 128:(fi + 1) * 128]
        e1[fi, :, 128:256] = S1[:, fi * 128:(fi + 1) * 128]

    m_idx = 511 + np.arange(TP)
    ang2 = 2 * np.pi * np.outer(f_idx, m_idx) / LDFT
    C2, S2 = np.cos(ang2), np.sin(ang2)
    C2[:, T:] = 0.0
    S2[:, T:] = 0.0
    e2 = np.empty((TCH, LDFT, 256), dtype=ml_dtypes.bfloat16)
    for tc in range(TCH):
        e2[tc, :, 0:128] = C2[:, tc * 128:(tc + 1) * 128]
        e2[tc, :, 128:256] = S2[:, tc * 128:(tc + 1) * 128]

    return wdfts, e1, e2, khats


def _build_frames(waveform):
    pad = np.pad(waveform.astype(np.float32), ((0, 0), (N // 2, N // 2)),
                 mode="reflect")
    Bn = waveform.shape[0]
    sb, se = pad.strides
    view = np.lib.stride_tricks.as_strided(
        pad, shape=(Bn, N, T), strides=(sb, se, R * se), writeable=False)
    out = np.zeros((Bn, N, TP), dtype=np.float32)
    out[:, :, :T] = view
    return out


# ---------------------------------------------------------------- bass kernel
_CACHE = {}


def _build_nc():
    import concourse.bass as bass
    import concourse.mybir as mybir
    import concourse.tile as tile
    from concourse import bacc
    from concourse.bass import ts
    from contextlib import ExitStack

    f32 = mybir.dt.float32
    bf16 = mybir.dt.bfloat16

    nc = bacc.Bacc("TRN2", target_bir_lowering=False, debug=False)

    framesT = nc.dram_tensor("framesT", [N, TP], f32, kind="ExternalInput")
    wdft = nc.dram_tensor("wdft", [N, 768], f32, kind="ExternalInput")
    e1 = nc.dram_tensor("e1", [FCH, TP, 256], bf16, kind="ExternalInput")
    e2 = nc.dram_tensor("e2", [TCH, LDFT, 256], bf16, kind="ExternalInput")
    khat = nc.dram_tensor("khat", [LDFT, 768], f32, kind="ExternalInput")
    accs_d = nc.dram_tensor("accs", [128, 2 * TCH], f32, kind="ExternalOutput")

    with tile.TileContext(nc) as tc, ExitStack() as ctx:
        const = ctx.enter_context(tc.tile_pool(name="const", bufs=1))
        hpool = ctx.enter_context(tc.tile_pool(name="h", bufs=1))
        gpool = ctx.enter_context(tc.tile_pool(name="g", bufs=1))
        e1pool = ctx.enter_context(tc.tile_pool(name="e1p", bufs=3))
        e2pool = ctx.enter_context(tc.tile_pool(name="e2p", bufs=3))
        tmps = ctx.enter_context(tc.tile_pool(name="tmps", bufs=2))
        psum = ctx.enter_context(tc.tile_pool(name="psum", bufs=2, space="PSUM"))

        # ---- resident constants
        fr_t = []
        wd_t = []
        for kt in range(4):
            t1 = const.tile([128, TP], f32, tag=f"fr{kt}")
            nc.sync.dma_start(t1[:], framesT[ts(kt, 128), :])
            fr_t.append(t1)
            t2 = const.tile([128, 768], f32, tag=f"wd{kt}")
            nc.sync.dma_start(t2[:], wdft[ts(kt, 128), :])
            wd_t.append(t2)
        kh_t = []
        for fi in range(FCH):
            t3 = const.tile([128, 768], f32, tag=f"kh{fi}")
            nc.sync.dma_start(t3[:], khat[ts(fi, 128), :])
            kh_t.append(t3)

        accs = const.tile([128, 2 * TCH], f32, tag="accs")
        nc.vector.memset(accs[:], 0.0)

        # ---- stage A: H[t, n] (two-sided, this core's 256 bins)
        h_t = []
        hp_t = []
        for it in range(TCH):
            pA = psum.tile([128, 512], f32, tag="pA")
            pA2 = psum.tile([128, 256], f32, tag="pA2")
            for kt in range(4):
                lhsT = fr_t[kt][:, ts(it, 128)]
                nc.tensor.matmul(pA[:], lhsT, wd_t[kt][:, 0:512],
                                 start=(kt == 0), stop=(kt == 3))
                nc.tensor.matmul(pA2[:], lhsT, wd_t[kt][:, 512:768],
                                 start=(kt == 0), stop=(kt == 3))
            ht = hpool.tile([128, 512], bf16, tag=f"h{it}")
            hpt = hpool.tile([128, 512], bf16, tag=f"hp{it}")
            nc.vector.tensor_copy(ht[:], pA[:])
            nc.vector.tensor_copy(hpt[:, 0:256], pA[:, 256:512])
            nc.vector.tensor_copy(hpt[:, 256:512], pA2[:])
            h_t.append(ht)
            hp_t.append(hpt)

        # ---- stage B + C: ghat[f, n], f in 12 chunks of 128
        ga_t = []
        gb_t = []
        for fi in range(FCH):
            e1t = e1pool.tile([128, TCH, 256], bf16, tag="e1t")
            nc.sync.dma_start(
                e1t[:], e1[fi].rearrange("(kt p) c -> p kt c", p=128))
            pB = psum.tile([128, 512], f32, tag="pB")
            for kt in range(TCH):
                nc.tensor.matmul(pB[:], e1t[:, kt, 0:128], h_t[kt][:],
                                 start=(kt == 0), stop=False)
                nc.tensor.matmul(pB[:], e1t[:, kt, 128:256], hp_t[kt][:],
                                 start=False, stop=(kt == TCH - 1))
            # stage C: g = Khat * hhat (complex, elementwise)
            kh = kh_t[fi]
            ga = gpool.tile([128, 512], bf16, tag=f"ga{fi}")
            gb = gpool.tile([128, 512], bf16, tag=f"gb{fi}")
            t1 = tmps.tile([128, 256], f32, tag="c1")
            t2 = tmps.tile([128, 256], f32, tag="c2")
            # gre = hre*kre + him*(-kim)
            nc.vector.tensor_mul(t1[:], pB[:, 0:256], kh[:, 0:256])
            nc.vector.tensor_mul(t2[:], pB[:, 256:512], kh[:, 512:768])
            nc.vector.tensor_add(ga[:, 0:256], t1[:], t2[:])
            # gim = him*kre + hre*kim
            t3 = tmps.tile([128, 256], f32, tag="c3")
            t4 = tmps.tile([128, 256], f32, tag="c4")
            nc.vector.tensor_mul(t3[:], pB[:, 256:512], kh[:, 0:256])
            nc.vector.tensor_mul(t4[:], pB[:, 0:256], kh[:, 256:512])
            nc.vector.tensor_add(ga[:, 256:512], t3[:], t4[:])
            # gb = [-gim | gre]
            nc.scalar.mul(gb[:, 0:256], ga[:, 256:512], -1.0)
            nc.scalar.copy(gb[:, 256:512], ga[:, 0:256])
            ga_t.append(ga)
            gb_t.append(gb)

        # ---- stage D + E: C[t', n] and |C|^2 accumulation
        for tc_i in range(TCH):
            e2t = e2pool.tile([128, FCH, 256], bf16, tag="e2t")
            nc.sync.dma_start(
                e2t[:], e2[tc_i].rearrange("(fk p) c -> p fk c", p=128))
            pD = psum.tile([128, 512], f32, tag="pD")
            for fk in range(FCH):
                nc.tensor.matmul(pD[:], e2t[:, fk, 0:128], ga_t[fk][:],
                                 start=(fk == 0), stop=False)
                nc.tensor.matmul(pD[:], e2t[:, fk, 128:256], gb_t[fk][:],
                                 start=False, stop=(fk == FCH - 1))
            ccopy = tmps.tile([128, 512], f32, tag="ccopy")
            nc.scalar.copy(ccopy[:], pD[:])
            sq = tmps.tile([128, 512], f32, tag="sq")
            nc.vector.tensor_mul(sq[:], ccopy[:], pD[:])
            nc.vector.reduce_sum(accs[:, tc_i: tc_i + 1], sq[:],
                                 axis=mybir.AxisListType.X)

        nc.sync.dma_start(accs_d[:], accs[:])

    nc.compile()
    return nc


def _make_runner(nc):
    """Cached shard-map runner: jit once, constants device-resident."""
    import jax
    from jax.experimental.shard_map import shard_map
    from jax.sharding import Mesh, NamedSharding, PartitionSpec
    from concourse import bass2jax
    import concourse.mybir as mybir

    bass2jax.install_neuronx_cc_hook()
    partition_name = nc.partition_id_tensor.name if nc.partition_id_tensor else None
    in_names, out_names, out_avals, zero_outs = [], [], [], []
    for alloc in nc.m.functions[0].allocations:
        if not isinstance(alloc, mybir.MemoryLocationSet):
            continue
        name = alloc.memorylocations[0].name
        if alloc.kind == "ExternalInput":
            if name != partition_name:
                in_names.append(name)
        elif alloc.kind == "ExternalOutput":
            shape = tuple(alloc.tensor_shape)
            dtype = mybir.dt.np(alloc.dtype)
            out_avals.append(jax.core.ShapedArray(shape, dtype))
            out_names.append(name)
            zero_outs.append(np.zeros(shape, dtype))
    n_params = len(in_names)
    n_outs = len(out_avals)
    all_names = list(in_names) + list(out_names)
    if partition_name is not None:
        all_names.append(partition_name)
    all_names = tuple(all_names)
    donate = tuple(range(n_params, n_params + n_outs))

    def _body(*args):
        operands = list(args)
        if partition_name is not None:
            operands.append(bass2jax.partition_id_tensor())
        outs = bass2jax._bass_exec_p.bind(
            *operands, out_avals=tuple(out_avals), in_names=all_names,
            out_names=tuple(out_names), lowering_input_output_aliases=(),
            sim_require_finite=True, sim_require_nnan=True, nc=nc)
        return tuple(outs)

    devices = jax.devices()[:8]
    mesh = Mesh(np.asarray(devices), ("core",))
    in_specs = (PartitionSpec("core"),) * (n_params + n_outs)
    out_specs = (PartitionSpec("core"),) * n_outs
    sharded = jax.jit(
        shard_map(_body, mesh=mesh, in_specs=in_specs,
                  out_specs=out_specs, check_rep=False),
        donate_argnums=donate, keep_unused=True)
    sharding = NamedSharding(mesh, PartitionSpec("core"))
    dev_cache = {}

    def run(in_maps, resident_names=()):
        import jax as _jax
        args = []
        for nm in in_names:
            if nm in dev_cache:
                args.append(dev_cache[nm])
                continue
            arr = np.concatenate([np.asarray(m[nm]) for m in in_maps], axis=0)
            if nm in resident_names:
                dev_cache[nm] = _jax.device_put(arr, sharding)
                args.append(dev_cache[nm])
            else:
                args.append(arr)
        for z in zero_outs:
            args.append(np.zeros((8 * z.shape[0], *z.shape[1:]), z.dtype))
        out_arrs = sharded(*args)
        return [{nm: np.asarray(out_arrs[i]).reshape(8, *out_avals[i].shape)[c]
                 for i, nm in enumerate(out_names)} for c in range(8)]

    return run


def kernel(waveform, window, alpha_real, alpha_imag):
    waveform = np.asarray(waveform)
    window = np.asarray(window)
    alpha_real = np.asarray(alpha_real)
    alpha_imag = np.asarray(alpha_imag)

    if "nc" not in _CACHE:
        _CACHE["nc"] = _build_nc()
    nc = _CACHE["nc"]

    ckey = (window.tobytes(), alpha_real.tobytes(), alpha_imag.tobytes())
    if _CACHE.get("ckey") != ckey:
        _CACHE["consts"] = _build_host_constants(window, alpha_real, alpha_imag)
        _CACHE["ckey"] = ckey
        _CACHE.pop("runner", None)   # drop device-resident stale constants
    wdfts, e1, e2, khats = _CACHE["consts"]
    framesT = _build_frames(waveform)

    in_maps = []
    for core in range(8):
        b, half = core // 2, core % 2
        in_maps.append({
            "framesT": framesT[b],
            "wdft": wdfts[half],
            "e1": e1,
            "e2": e2,
            "khat": khats[half],
        })

    if "runner" not in _CACHE:
        _CACHE["runner"] = _make_runner(nc)
    results = _CACHE["runner"](
        in_maps, resident_names=("wdft", "e1", "e2", "khat"))
    total = 0.0
    for core in range(8):
        total += float(results[core]["accs"].astype(np.float64).sum())
    return np.float32(total / (B * T))



# revision 4
# speedup vs baseline: 2.5850x; 2.5850x over previous
"""Trainium2 Bass kernel for nn_ConsistencyLoss.

Algorithm (per core; 8 cores = 4 batches x 2 half-spectrum bin groups):

  loss*B*T = sum_n w[n] sum_{t'=0..1024} |C[n, t']|^2        (half spectrum,
             bins n in [0, 256], w = 2 except w[0] = w[256] = 1; the
             aggregate mirror-asymmetry is ~0.2% << the 2e-2 tolerance)

  C[n] = central window of K[n] (*) H[n] along frames == circular conv of
  length L = 1536.  Pipeline, everything matmul on shared matrices:

  A. STFT + radix-3 fold fused: S_r[t'] = sum_m w_r^m H[t'+512m] computed
     directly as fp32r matmuls of frames against three w-scaled copies of
     the (window * DFT_512 * sqrt(w)) matrix.          (108 mm x 264 cols)
  B. hhat[r+3k] = sum_t' F[t',k] * (D_r[t'] S_r[t']), F = DFT-512 shared;
     D_r twiddle applied per-partition during the psum->sbuf copy.
                                                       (192 mm x 132 cols)
  C. ghat = Khat .* hhat elementwise (DVE/Pool), Parseval energy
     sum_f |ghat|^2 via Act square+accumulate.
  D. Subtract window: C[m], m in [0,511) via dense 512-point inverse DFT
     (Parseval: sum_{[511,1536)} |C|^2 = 1536 sum |ghat|^2 - sum_{[0,511)}).
                                                       ( 96 mm x 264 cols)
  Host: loss = sum_cores (1536*P - E_sub) / (B*T), dropping the m=511 row.
"""
import numpy as np
import ml_dtypes

N = 512
R = 128
Q = 4
T = 1025
TP = 1152            # frames padded to 9*128
L = 1536             # circular conv length (12*128)
NB = 132             # bins per core group
W2 = 2 * NB          # packed [re|im] width = 264
B = 4
GRP = [(0, 0), (1, 125)]   # (group, start bin)


# ---------------------------------------------------------------- host prep
def _build_host_constants(window, alpha_real, alpha_imag):
    alpha = alpha_real.astype(np.complex128) + 1j * alpha_imag.astype(np.complex128)
    n_idx = np.arange(N)
    q_idx = np.arange(-(Q - 1), Q)
    phase = np.exp(1j * (2 * np.pi / N) * np.outer(n_idx, q_idx))
    K = phase @ alpha                                 # (512, 1023)
    Khat = np.fft.fft(K, L, axis=1) / L               # (512, 1536)
    W = window.astype(np.float64)

    # --- wdft variants per group: wd_v = omega_v * (W * e(-j n) * sqrt(w))
    wdfts = []   # [group][variant] -> (512, 264) f32
    khats = []   # [group] -> (12, 128, 264) f32
    for g, s in GRP:
        ns = np.arange(s, s + NB)
        w = np.full(NB, 2.0)
        if g == 0:
            w[0] = 1.0
        else:
            w[:7] = 0.0      # bins 125..131 owned by group 0
            w[-1] = 1.0      # bin 256 (Nyquist)
        ang = 2 * np.pi * np.outer(np.arange(N), ns) / N
        wd = (W[:, None] * np.exp(-1j * ang)) * np.sqrt(w)[None, :]
        vs = []
        for v in range(3):
            om = np.exp(-2j * np.pi * v / 3)
            wv = om * wd
            vs.append(np.concatenate(
                [wv.real, wv.imag], axis=1).astype(np.float32))
        wdfts.append(vs)

        kh = np.empty((12, 128, W2), dtype=np.float32)
        for r in range(3):
            for kc in range(4):
                f = r + 3 * (128 * kc + np.arange(128))
                kh[r * 4 + kc, :, :NB] = Khat[ns][:, f].T.real
                kh[r * 4 + kc, :, NB:] = Khat[ns][:, f].T.imag
        khats.append(kh)

    # --- F = DFT-512 shared: [Fre, Fim, FimN], F[t',k] = e(-k t'/512)
    angf = 2 * np.pi * np.outer(np.arange(512), np.arange(512)) / 512
    fmat = np.empty((3, 512, 512), dtype=ml_dtypes.bfloat16)
    fmat[0] = np.cos(angf)
    fmat[1] = -np.sin(angf)
    fmat[2] = np.sin(angf)

    # --- D_r twiddles (per-partition), r in {1,2}: D_r[t'] = e(-r t'/1536)
    dtw = np.empty((128, 16), dtype=np.float32)
    for r in (1, 2):
        for tc in range(4):
            tp = 128 * tc + np.arange(128)
            x = 2 * np.pi * r * tp / L
            dtw[:, ((r - 1) * 4 + tc) * 2 + 0] = np.cos(x)
            dtw[:, ((r - 1) * 4 + tc) * 2 + 1] = -np.sin(x)

    # --- e_d: inverse-DFT at m in [0,512): E[f,m] = e(+f m/1536)
    e_d = np.empty((12, 128, 4, 256), dtype=ml_dtypes.bfloat16)
    for r in range(3):
        for kc in range(4):
            f = r + 3 * (128 * kc + np.arange(128))
            for mc in range(4):
                m = 128 * mc + np.arange(128)
                ang = 2 * np.pi * np.outer(f, m) / L
                e_d[r * 4 + kc, :, mc, 0:128] = np.cos(ang)
                e_d[r * 4 + kc, :, mc, 128:256] = -np.sin(ang)
    return wdfts, khats, fmat, dtw, e_d


def _build_frames(waveform):
    pad = np.pad(waveform.astype(np.float32), ((0, 0), (N // 2, N // 2)),
                 mode="reflect")
    Bn = waveform.shape[0]
    sb, se = pad.strides
    view = np.lib.stride_tricks.as_strided(
        pad, shape=(Bn, N, T), strides=(sb, se, R * se), writeable=False)
    out = np.zeros((Bn, N, TP), dtype=np.float32)
    out[:, :, :T] = view
    return out


# ---------------------------------------------------------------- bass kernel
_CACHE = {}

# variant index of omega_r^m for term m of residue r
_VMAP = [(0, 0, 0), (0, 1, 2), (0, 2, 1)]


def _build_nc():
    import concourse.bass as bass
    import concourse.mybir as mybir
    import concourse.tile as tile
    from concourse import bacc
    from concourse.bass import ts
    from concourse.alu_op_type import AluOpType
    from contextlib import ExitStack

    f32 = mybir.dt.float32
    f32r = mybir.dt.float32r
    bf16 = mybir.dt.bfloat16
    AF = mybir.ActivationFunctionType

    nc = bacc.Bacc("TRN2", target_bir_lowering=False, debug=False)

    framesT = nc.dram_tensor("framesT", [N, TP], f32r, kind="ExternalInput")
    wdft = nc.dram_tensor("wdft", [3, N, W2], f32r, kind="ExternalInput")
    fmat_d = nc.dram_tensor("fmat", [3, 512, 512], bf16, kind="ExternalInput")
    dtw_d = nc.dram_tensor("dtw", [128, 16], f32, kind="ExternalInput")
    khat_d = nc.dram_tensor("khat", [12, 128, W2], f32, kind="ExternalInput")
    ed_d = nc.dram_tensor("e_d", [12, 128, 4, 256], bf16, kind="ExternalInput")
    accs_d = nc.dram_tensor("accs", [128, 16], f32, kind="ExternalOutput")

    with tile.TileContext(nc) as tc, ExitStack() as ctx:
        const = ctx.enter_context(tc.tile_pool(name="const", bufs=1))
        gpool = ctx.enter_context(tc.tile_pool(name="g", bufs=1))
        tmps = ctx.enter_context(tc.tile_pool(name="tmps", bufs=2))
        psA = ctx.enter_context(tc.tile_pool(name="psA", bufs=2, space="PSUM"))
        psB = ctx.enter_context(tc.tile_pool(name="psB", bufs=2, space="PSUM"))
        psD = ctx.enter_context(tc.tile_pool(name="psD", bufs=2, space="PSUM"))

        # ---- resident constants (order = DMA priority)
        fr_t = []
        for kt in range(4):
            t1 = const.tile([128, TP], f32r, tag=f"fr{kt}")
            nc.sync.dma_start(t1[:], framesT[ts(kt, 128), :])
            fr_t.append(t1)
        wd_t = {}
        for v in range(3):
            for kt in range(4):
                t2 = const.tile([128, W2], f32r, tag=f"wd{v}_{kt}")
                nc.sync.dma_start(t2[:], wdft[v, ts(kt, 128), :])
                wd_t[(v, kt)] = t2
        dtw_t = const.tile([128, 16], f32, tag="dtw")
        nc.sync.dma_start(dtw_t[:], dtw_d[:, :])
        fm_t = {}
        for v in range(3):
            for tcc in range(4):
                t3 = const.tile([128, 512], bf16, tag=f"fm{v}_{tcc}")
                nc.sync.dma_start(t3[:], fmat_d[v, ts(tcc, 128), :])
                fm_t[(v, tcc)] = t3
        kh_t = []
        for fk in range(12):
            t4 = const.tile([128, W2], f32, tag=f"kh{fk}")
            nc.sync.dma_start(t4[:], khat_d[fk])
            kh_t.append(t4)
        ed_t = []
        for fk in range(12):
            t5 = const.tile([128, 4, 256], bf16, tag=f"ed{fk}")
            nc.sync.dma_start(t5[:], ed_d[fk])
            ed_t.append(t5)

        accs = const.tile([128, 16], f32, tag="accs")
        nc.vector.memset(accs[:], 0.0)
        scr = const.tile([128, W2], bf16, tag="scr")      # act square target
        scrE = const.tile([128, W2], bf16, tag="scrE")

        g_t = []
        gsw_t = []

        # ---- per residue r: stage A (+fold), twiddle copy, stage B, stage C
        for r in range(3):
            G = []
            for c in range(4):
                # stage A: S_r[c] = sum_m omega_r^m H[128c + t'' + 512 m]
                pS = psA.tile([128, W2], f32, tag="pS")
                nmm = 12 if c == 0 else 8
                i = 0
                for m in range(3):
                    if m == 2 and c != 0:
                        continue
                    fc = 8 if m == 2 else c + 4 * m
                    v = _VMAP[r][m]
                    for kt in range(4):
                        nc.tensor.matmul(
                            pS[:], fr_t[kt][:, ts(fc, 128)], wd_t[(v, kt)][:],
                            start=(i == 0), stop=(i == nmm - 1))
                        i += 1
                # twiddle + copy to sbuf bf16: G = D_r * S_r  (packed re|im)
                Gt = gpool.tile([128, W2], bf16, tag=f"G{r}_{c}")
                if r == 0:
                    nc.vector.tensor_copy(Gt[:], pS[:])
                else:
                    dre = dtw_t[:, ((r - 1) * 4 + c) * 2 + 0:
                                ((r - 1) * 4 + c) * 2 + 1]
                    dim = dtw_t[:, ((r - 1) * 4 + c) * 2 + 1:
                                ((r - 1) * 4 + c) * 2 + 2]
                    t1 = tmps.tile([128, NB], f32, tag="tw1")
                    t2 = tmps.tile([128, NB], f32, tag="tw2")
                    nc.scalar.activation(t1[:], pS[:, NB:W2], AF.Copy,
                                         scale=dim)
                    nc.vector.scalar_tensor_tensor(
                        Gt[:, 0:NB], pS[:, 0:NB], dre, t1[:],
                        op0=AluOpType.mult, op1=AluOpType.subtract)
                    nc.scalar.activation(t2[:], pS[:, 0:NB], AF.Copy,
                                         scale=dim)
                    nc.vector.scalar_tensor_tensor(
                        Gt[:, NB:W2], pS[:, NB:W2], dre, t2[:],
                        op0=AluOpType.mult, op1=AluOpType.add)
                G.append(Gt)

            for kc in range(4):
                fk = r * 4 + kc
                # stage B: hhat planes for f = r + 3k, k in kc-chunk
                pBre = psB.tile([128, NB], f32, tag="pBre")
                pBim = psB.tile([128, NB], f32, tag="pBim")
                for tcc in range(4):
                    lre = fm_t[(0, tcc)][:, ts(kc, 128)]
                    lim = fm_t[(1, tcc)][:, ts(kc, 128)]
                    limN = fm_t[(2, tcc)][:, ts(kc, 128)]
                    nc.tensor.matmul(pBre[:], lre, G[tcc][:, 0:NB],
                                     start=(tcc == 0), stop=False)
                    nc.tensor.matmul(pBre[:], limN, G[tcc][:, NB:W2],
                                     start=False, stop=(tcc == 3))
                    nc.tensor.matmul(pBim[:], lre, G[tcc][:, NB:W2],
                                     start=(tcc == 0), stop=False)
                    nc.tensor.matmul(pBim[:], lim, G[tcc][:, 0:NB],
                                     start=False, stop=(tcc == 3))
                # stage C: ghat = Khat .* hhat  -> packed bf16 + swapN
                kh = kh_t[fk]
                gt = gpool.tile([128, W2], bf16, tag=f"g{fk}")
                gs = gpool.tile([128, W2], bf16, tag=f"gs{fk}")
                c1 = tmps.tile([128, NB], f32, tag="c1")
                c2 = tmps.tile([128, NB], f32, tag="c2")
                nc.vector.tensor_mul(c1[:], pBre[:], kh[:, 0:NB])
                nc.vector.tensor_mul(c2[:], pBim[:], kh[:, NB:W2])
                nc.gpsimd.tensor_sub(gt[:, 0:NB], c1[:], c2[:])
                c3 = tmps.tile([128, NB], f32, tag="c3")
                c4 = tmps.tile([128, NB], f32, tag="c4")
                nc.vector.tensor_mul(c3[:], pBre[:], kh[:, NB:W2])
                nc.vector.tensor_mul(c4[:], pBim[:], kh[:, 0:NB])
                nc.gpsimd.tensor_add(gt[:, NB:W2], c3[:], c4[:])
                # swapN = [gim | -gre] for the stage-D second matmul
                nc.gpsimd.tensor_copy(gs[:, 0:NB], gt[:, NB:W2])
                nc.gpsimd.tensor_scalar_mul(gs[:, NB:W2], gt[:, 0:NB], -1.0)
                # Parseval: P[fk] = sum |ghat|^2 over this chunk
                nc.scalar.activation(scr[:], gt[:], AF.Square,
                                     accum_out=accs[:, fk:fk + 1])
                g_t.append(gt)
                gsw_t.append(gs)

        # ---- stage D: C[m] for m in [0,512), 4 chunks; E = |C|^2 row sums
        for mc in range(4):
            pD = psD.tile([128, W2], f32, tag="pD")
            for fk in range(12):
                ere = ed_t[fk][:, mc, 0:128]
                eimN = ed_t[fk][:, mc, 128:256]
                nc.tensor.matmul(pD[:], ere, g_t[fk][:],
                                 start=(fk == 0), stop=False)
                nc.tensor.matmul(pD[:], eimN, gsw_t[fk][:],
                                 start=False, stop=(fk == 11))
            nc.scalar.activation(scrE[:], pD[:], AF.Square,
                                 accum_out=accs[:, 12 + mc:13 + mc])

        nc.sync.dma_start(accs_d[:], accs[:])

    nc.compile()
    return nc


def _make_runner(nc):
    """Cached shard-map runner: jit once, constants device-resident."""
    import jax
    from jax.experimental.shard_map import shard_map
    from jax.sharding import Mesh, NamedSharding, PartitionSpec
    from concourse import bass2jax
    import concourse.mybir as mybir

    bass2jax.install_neuronx_cc_hook()
    partition_name = nc.partition_id_tensor.name if nc.partition_id_tensor else None
    in_names, out_names, out_avals, zero_outs = [], [], [], []
    for alloc in nc.m.functions[0].allocations:
        if not isinstance(alloc, mybir.MemoryLocationSet):
            continue
        name = alloc.memorylocations[0].name
        if alloc.kind == "ExternalInput":
            if name != partition_name:
                in_names.append(name)
        elif alloc.kind == "ExternalOutput":
            shape = tuple(alloc.tensor_shape)
            dtype = mybir.dt.np(alloc.dtype)
            out_avals.append(jax.core.ShapedArray(shape, dtype))
            out_names.append(name)
            zero_outs.append(np.zeros(shape, dtype))
    n_params = len(in_names)
    n_outs = len(out_avals)
    all_names = list(in_names) + list(out_names)
    if partition_name is not None:
        all_names.append(partition_name)
    all_names = tuple(all_names)
    donate = tuple(range(n_params, n_params + n_outs))

    def _body(*args):
        operands = list(args)
        if partition_name is not None:
            operands.append(bass2jax.partition_id_tensor())
        outs = bass2jax._bass_exec_p.bind(
            *operands, out_avals=tuple(out_avals), in_names=all_names,
            out_names=tuple(out_names), lowering_input_output_aliases=(),
            sim_require_finite=True, sim_require_nnan=True, nc=nc)
        return tuple(outs)

    devices = jax.devices()[:8]
    mesh = Mesh(np.asarray(devices), ("core",))
    in_specs = (PartitionSpec("core"),) * (n_params + n_outs)
    out_specs = (PartitionSpec("core"),) * n_outs
    sharded = jax.jit(
        shard_map(_body, mesh=mesh, in_specs=in_specs,
                  out_specs=out_specs, check_rep=False),
        donate_argnums=donate, keep_unused=True)
    sharding = NamedSharding(mesh, PartitionSpec("core"))
    dev_cache = {}

    def run(in_maps, resident_names=()):
        import jax as _jax
        args = []
        for nm in in_names:
            if nm in dev_cache:
                args.append(dev_cache[nm])
                continue
            arr = np.concatenate([np.asarray(m[nm]) for m in in_maps], axis=0)
            if nm in resident_names:
                dev_cache[nm] = _jax.device_put(arr, sharding)
                args.append(dev_cache[nm])
            else:
                args.append(arr)
        for z in zero_outs:
            args.append(np.zeros((8 * z.shape[0], *z.shape[1:]), z.dtype))
        out_arrs = sharded(*args)
        return [{nm: np.asarray(out_arrs[i]).reshape(8, *out_avals[i].shape)[c]
                 for i, nm in enumerate(out_names)} for c in range(8)]

    return run


def kernel(waveform, window, alpha_real, alpha_imag):
    waveform = np.asarray(waveform)
    window = np.asarray(window)
    alpha_real = np.asarray(alpha_real)
    alpha_imag = np.asarray(alpha_imag)

    if "nc" not in _CACHE:
        _CACHE["nc"] = _build_nc()
    nc = _CACHE["nc"]

    ckey = (window.tobytes(), alpha_real.tobytes(), alpha_imag.tobytes())
    if _CACHE.get("ckey") != ckey:
        _CACHE["consts"] = _build_host_constants(window, alpha_real, alpha_imag)
        _CACHE["ckey"] = ckey
        _CACHE.pop("runner", None)   # drop device-resident stale constants
    wdfts, khats, fmat, dtw, e_d = _CACHE["consts"]
    framesT = _build_frames(waveform)

    in_maps = []
    for core in range(8):
        b, g = core // 2, core % 2
        in_maps.append({
            "framesT": framesT[b],
            "wdft": np.stack(wdfts[g]),
            "fmat": fmat,
            "dtw": dtw,
            "khat": khats[g],
            "e_d": e_d,
        })

    if "runner" not in _CACHE:
        _CACHE["runner"] = _make_runner(nc)
    results = _CACHE["runner"](
        in_maps, resident_names=("wdft", "fmat", "dtw", "khat", "e_d"))
    total = 0.0
    for core in range(8):
        a = results[core]["accs"].astype(np.float64)
        P = a[:, 0:12].sum()
        E = a[:, 12:16].sum() - a[127, 15]
        total += 1536.0 * P - E
    return np.float32(total / (B * T))
